# revision 8
# baseline (speedup 1.0000x reference)
"""BEV->RV scatter-max kernel for 8 Trainium2 NeuronCores.

Strategy: shard by (batch, BEV-quadrant). Each BEV grid quadrant maps to a
disjoint RV column range (phi quadrants), so the 8 cores (2 batches x 4
quadrants) produce disjoint output slabs.

Layout (host, static/data-independent): pixels of each quadrant are grouped by
RV column into segments of <=SEG_K slots, padded, and placed in a fixed
[128 partitions x F free] layout. Per-pixel static tables (row_low, and the
30-entry row_high profile H[z]) are precomputed host-side with float32
arithmetic replicating the reference exactly.

Device: computes row_high by 30-plane select on z, builds per-row masks, and
does 64 x 32 masked segmented max-reduces. Host reduces segments to columns.
"""
import math
import sys

sys.path.insert(0, "/opt/trn_rl_repo")

import numpy as np

H_B, W_B = 512, 512
H_R, W_R = 64, 2048
Z_MIN, Z_MAX = -4.0, 2.0
Z_BINS = 30
Z_LOW = -1.73
PHI_MIN, PHI_MAX = -math.pi, math.pi
THETA_MIN, THETA_MAX = math.radians(-25.0), math.radians(3.0)
XMIN, XMAX, YMIN, YMAX = -50.0, 50.0, -50.0, 50.0

C = 32
B = 2
NEG = np.float32(-1.0e30)

SEG_K = 64          # slots per segment
SEG_PP = 10         # segments per partition
P = 128
F = SEG_K * SEG_PP  # free dim per partition (576)
NSEG = P * SEG_PP   # 1152 segment capacity

_QUADS = {
    0: (slice(0, 256), slice(0, 256)),
    1: (slice(0, 256), slice(256, 512)),
    2: (slice(256, 512), slice(0, 256)),
    3: (slice(256, 512), slice(256, 512)),
}


def _geometry_f32():
    """Replicates reference._geometry() numpy-f32 semantics exactly."""
    y = np.linspace(YMAX, YMIN, H_B, dtype=np.float32)
    x = np.linspace(XMIN, XMAX, W_B, dtype=np.float32)
    yg, xg = np.meshgrid(y, x, indexing="ij")
    rho = np.sqrt((xg * xg + yg * yg).astype(np.float32)).astype(np.float32)
    phi = np.arctan2(yg, xg)
    theta_low = np.arctan2(np.float32(Z_LOW), rho)
    row_low = np.clip(
        np.rint((THETA_MAX - theta_low) / (THETA_MAX - THETA_MIN) * (H_R - 1)),
        0, H_R - 1,
    ).astype(np.int32)
    col = np.clip(
        np.rint((phi - PHI_MIN) / (PHI_MAX - PHI_MIN) * (W_R - 1)), 0, W_R - 1
    ).astype(np.int32)
    return rho, row_low, col


def _row_high_table(rho_flat):
    """H[z, n]: row_high for each z bin, f32 ops replicating the reference."""
    dz = (Z_MAX - Z_MIN) / Z_BINS
    zc = (np.arange(Z_BINS).astype(np.float32) * np.float32(dz)
          + np.float32(Z_MIN + dz / 2)).astype(np.float32)
    th = np.arctan2(zc[:, None].astype(np.float32), rho_flat[None, :]).astype(np.float32)
    a = (np.float32(THETA_MAX) - th).astype(np.float32)
    b = (a / np.float32(THETA_MAX - THETA_MIN)).astype(np.float32)
    cexpr = (b * np.float32(H_R - 1)).astype(np.float32)
    return np.clip(np.rint(cexpr), 0, H_R - 1).astype(np.int32)  # [30, N]


class _Static:
    pass


_S = None


def _build_static():
    global _S
    if _S is not None:
        return _S
    S = _Static()
    rho, row_low, col = _geometry_f32()
    S.quads = []
    for q in range(4):
        si, sj = _QUADS[q]
        qcol = col[si, sj].ravel()
        qrho = rho[si, sj].ravel().astype(np.float32)
        qrl = row_low[si, sj].ravel()
        # flat pixel indices within the full 512x512 grid for this quadrant
        ii, jj = np.meshgrid(np.arange(si.start, si.stop), np.arange(sj.start, sj.stop),
                             indexing="ij")
        qpix = (ii * W_B + jj).ravel()

        order = np.argsort(qcol, kind="stable")
        c0, c1 = int(qcol.min()), int(qcol.max())
        ncols = c1 - c0 + 1
        counts = np.bincount(qcol - c0, minlength=ncols)

        # build segments: (col, chunk) -> list of pixel positions (in quadrant order)
        seg_col = []       # column (global) per segment
        slot_src = []      # per segment: array of quadrant-pixel-indices
        pos = 0
        for ci in range(ncols):
            k = counts[ci]
            idxs = order[pos:pos + k]
            pos += k
            for off in range(0, k, SEG_K):
                chunk = idxs[off:off + SEG_K]
                seg_col.append(c0 + ci)
                slot_src.append(chunk)
        nseg = len(seg_col)
        assert nseg <= NSEG, (q, nseg)

        # slot layout: segment s -> partition s % 128, seg slot s // 128
        perm_dst = np.empty(0, dtype=np.int64)
        src_all = np.empty(0, dtype=np.int64)
        dst_all = []
        for s, chunk in enumerate(slot_src):
            p_, j_ = s % P, s // P
            base = p_ * F + j_ * SEG_K
            dst_all.append(base + np.arange(len(chunk)))
        dst_all = np.concatenate([d for d in dst_all]).astype(np.int64)
        src_all = np.concatenate([c for c in slot_src]).astype(np.int64)

        # static per-slot tables in layout order
        l_tab = np.full(P * F, 127.0, np.float32)
        l_tab[dst_all] = qrl[src_all].astype(np.float32)
        Hq = _row_high_table(qrho)  # [30, Nq]
        H_tab = np.full((Z_BINS, P * F), 127.0, np.float32)
        H_tab[:, dst_all] = Hq[:, src_all].astype(np.float32)

        # host->device value permutation: vals[c, dst_all] = bev[c, qpix[src_all]]
        S_q = _Static()
        S_q.c0, S_q.c1, S_q.ncols, S_q.nseg = c0, c1, ncols, nseg
        S_q.qpix_src = qpix[src_all]       # gather indices into flat 512*512
        S_q.dst = dst_all                  # scatter positions into [P*F]
        S_q.l_tab = l_tab.reshape(P, F)
        S_q.H_tab = H_tab.reshape(Z_BINS, P, F)
        S_q.seg_col = np.asarray(seg_col, np.int32)
        # for host reduction: segments are already produced in column order:
        # seg s covers seg_col[s]; reduceat starts where column changes
        S_q.col_starts = np.flatnonzero(
            np.r_[True, S_q.seg_col[1:] != S_q.seg_col[:-1]])
        S_q.uniq_cols = S_q.seg_col[S_q.col_starts]
        S.quads.append(S_q)
    _S = S
    return S


_NC = None


def _build_nc():
    global _NC
    if _NC is not None:
        return _NC
    import concourse.bass as bass
    import concourse.bacc as bacc
    import concourse.mybir as mybir
    from concourse.tile import TileContext

    nc = bacc.Bacc("TRN2", target_bir_lowering=False, debug=False, num_devices=8)
    vals = nc.declare_dram_parameter("vals", [C, P, F], mybir.dt.float32, isOutput=False)
    zb = nc.declare_dram_parameter("zb", [P, F], mybir.dt.float32, isOutput=False)
    ltab = nc.declare_dram_parameter("ltab", [P, F], mybir.dt.float32, isOutput=False)
    htab = nc.declare_dram_parameter("htab", [Z_BINS, P, F], mybir.dt.float32,
                                     isOutput=False)
    out = nc.declare_dram_parameter("out", [P, H_R * C * SEG_PP], mybir.dt.float32,
                                    isOutput=True)

    with TileContext(nc) as tc:
        with tc.tile_pool(name="sb", bufs=1) as pool, \
             tc.tile_pool(name="hplane", bufs=2) as hpool:
            v_t = []
            for c in range(C):
                vt = pool.tile([P, F], mybir.dt.float32, tag=f"v{c}")
                nc.sync.dma_start(out=vt[:], in_=vals[c])
                v_t.append(vt)
            zb_t = pool.tile([P, F], mybir.dt.float32, tag="zb")
            l_t = pool.tile([P, F], mybir.dt.float32, tag="l")
            nc.sync.dma_start(out=zb_t[:], in_=zb[:, :])
            nc.sync.dma_start(out=l_t[:], in_=ltab[:, :])

            h_t = pool.tile([P, F], mybir.dt.float32, tag="h")
            eq_t = pool.tile([P, F], mybir.dt.float32, tag="eq")
            nc.vector.memset(h_t[:], 0.0)
            for z in range(Z_BINS):
                hp = hpool.tile([P, F], mybir.dt.float32, tag="hp")
                nc.sync.dma_start(out=hp[:], in_=htab[z])
                # eq = (zb == z) * H_z ; h += eq
                nc.vector.tensor_scalar(
                    out=eq_t[:], in0=zb_t[:], scalar1=float(z), scalar2=None,
                    op0=mybir.AluOpType.is_equal)
                nc.vector.tensor_tensor(
                    out=eq_t[:], in0=eq_t[:], in1=hp[:], op=mybir.AluOpType.mult)
                nc.vector.tensor_tensor(
                    out=h_t[:], in0=h_t[:], in1=eq_t[:], op=mybir.AluOpType.add)

            s_t = pool.tile([P, F], mybir.dt.float32, tag="s")
            e_t = pool.tile([P, F], mybir.dt.float32, tag="e")
            nc.vector.tensor_tensor(out=s_t[:], in0=l_t[:], in1=h_t[:],
                                    op=mybir.AluOpType.min)
            nc.vector.tensor_tensor(out=e_t[:], in0=l_t[:], in1=h_t[:],
                                    op=mybir.AluOpType.max)

            out_t = pool.tile([P, H_R * C * SEG_PP], mybir.dt.float32, tag="out")
            mask_t = pool.tile([P, F], mybir.dt.float32, tag="mask")
            mb_t = pool.tile([P, F], mybir.dt.float32, tag="mb")
            tmp_t = pool.tile([P, F], mybir.dt.float32, tag="tmp")
            for r in range(H_R):
                fr = float(r)
                # mask = (s <= r)  * (e >= r) -> {0,1}; maskneg = mask*1e30-1e30
                nc.vector.tensor_scalar(
                    out=mask_t[:], in0=s_t[:], scalar1=fr, scalar2=None,
                    op0=mybir.AluOpType.is_le)
                nc.vector.tensor_scalar(
                    out=mb_t[:], in0=e_t[:], scalar1=fr, scalar2=None,
                    op0=mybir.AluOpType.is_ge)
                nc.vector.tensor_tensor(out=mask_t[:], in0=mask_t[:], in1=mb_t[:],
                                        op=mybir.AluOpType.mult)
                nc.vector.tensor_scalar(
                    out=mb_t[:], in0=mask_t[:], scalar1=float(1.0e30),
                    scalar2=float(-1.0e30), op0=mybir.AluOpType.mult,
                    op1=mybir.AluOpType.add)
                for c in range(C):
                    nc.vector.tensor_tensor(
                        out=tmp_t[:], in0=v_t[c][:], in1=mb_t[:],
                        op=mybir.AluOpType.add)
                    off = (r * C + c) * SEG_PP
                    nc.vector.tensor_reduce(
                        out=out_t[:, off:off + SEG_PP],
                        in_=tmp_t[:].rearrange("p (j k) -> p j k", k=SEG_K),
                        axis=mybir.AxisListType.X,
                        op=mybir.AluOpType.max)
            nc.sync.dma_start(out=out[:, :], in_=out_t[:])
    nc.compile()
    _NC = nc
    return nc


def kernel(bev_feat, bev_z_bin):
    from concourse.bass_utils import run_bass_kernel_spmd

    S = _build_static()
    nc = _build_nc()
    bev_feat = np.asarray(bev_feat, dtype=np.float32)
    bev_z_bin = np.asarray(bev_z_bin, dtype=np.int32)

    in_maps = []
    metas = []
    for core in range(8):
        b, q = core // 4, core % 4
        Sq = S.quads[q]
        flat = bev_feat[b].reshape(C, H_B * W_B)
        v = np.full((C, P * F), NEG, np.float32)
        v[:, Sq.dst] = flat[:, Sq.qpix_src]
        zflat = bev_z_bin[b, 0].reshape(H_B * W_B)
        z = np.zeros(P * F, np.float32)
        z[Sq.dst] = zflat[Sq.qpix_src].astype(np.float32)
        in_maps.append({
            "vals": v.reshape(C, P, F),
            "zb": z.reshape(P, F),
            "ltab": Sq.l_tab,
            "htab": Sq.H_tab,
        })
        metas.append((b, q))

    res = run_bass_kernel_spmd(nc, in_maps, list(range(8)))

    outp = np.zeros((B, C, H_R, W_R), np.float32)
    for core, (b, q) in enumerate(metas):
        Sq = S.quads[q]
        o = res.results[core]["out"].reshape(P, H_R, C, SEG_PP)
        # segment s lives at partition s % P, seg-slot s // P
        o = o.transpose(1, 2, 3, 0).reshape(H_R, C, NSEG)[:, :, :Sq.nseg]
        red = np.maximum.reduceat(o, Sq.col_starts, axis=2)
        block = np.where(red < -1.0e29, np.float32(0), red)
        # red/block is [H_R, C, ncols_used] -> outp[b] is [C, H_R, W_R]
        outp[b][:, :, Sq.uniq_cols] = block.transpose(1, 0, 2)
    return outp
